# revision 1
# baseline (speedup 1.0000x reference)
"""Trainium2 Bass kernel for nn_CharRNN: 2-layer MI-GRU + large vocab projection.

Strategy (8 NeuronCores, SPMD, no collectives):
  - The sequential GRU recurrence (T=50 steps, B=100) is replicated on all
    8 cores: per-step matmul time is weight-column bound (independent of B),
    so batch-sharding would not speed it up, and replication avoids any
    cross-core synchronization.
  - The output projection logits = out @ softmax_w + b ([5000, 8000], 160 MB)
    is sharded over the vocab axis: core i computes columns [i*1000, (i+1)*1000)
    and writes its own 20 MB slice (memory-bound part spread over 8 cores).

Layouts:
  - Gate/elementwise tensors: [B=100 partitions, features free].
  - Matmuls: out[B, N] = lhsT.T @ rhs with stationary lhsT = transposed
    activations [K=128 chunk, B] and moving rhs = weight columns (bf16,
    1 col/cycle). Hidden-state transposes done on the PE via identity matmul.
  - alpha/beta1/beta2/b are folded on the host:
      gate = sig((a*wx + b1) * (uh + b2/a) + (b - b1*b2/a))
    with W' = W*alpha baked into the uploaded weights and the remaining
    per-column constants (constant rows in this problem) applied as scalar
    biases fused into ACT activations / scalar_tensor_tensor ops.
"""

import os
import sys

sys.path.insert(0, "/opt/trn_rl_repo")

import ml_dtypes
import numpy as np

import concourse.bass as bass
import concourse.mybir as mybir
import concourse.tile as tile
from concourse.masks import make_identity

# ----------------------------------------------------------------------------
# Patch: the final SP Drain emitted by TileContext collects one semaphore wait
# per busy logical processor, but the walrus build in this container only
# lowers a limited number of sync-wait commands per CTRL instruction.  Split
# the waits across separate single-wait NoOps.
# ----------------------------------------------------------------------------
from concourse.vector_clock import ScopedClock
from bass_rust import SyncInfo

_MAXW = 1


def _patched_drain_and_barrier(self, tick_clock, wait_clock):
    nc = self.nc
    drain_inst = nc.sync.drain()
    wait_clock.add_sem_waits(
        drain_inst.ins, ScopedClock({None: tick_clock.global_clock})
    )
    si = drain_inst.ins.sync_info
    waits = list(si.on_wait) if si is not None else []
    if len(waits) > _MAXW:
        drain_inst.ins.sync_info = SyncInfo(
            on_wait=waits[:_MAXW], on_update=list(si.on_update)
        )
        for k in range(_MAXW, len(waits), _MAXW):
            nop = nc.sync.nop(nofuse=True)
            nop.ins.sync_info = SyncInfo(on_wait=waits[k : k + _MAXW], on_update=[])

    nc.all_engine_barrier()
    assert self.sems is not None
    popped = nc._tile_sem_poison_stack.pop()
    assert popped is self._sem_poison
    nc.clear_and_free_semaphores(list(self.sems.allocated().values()))
    nc.all_engine_barrier()


tile.TileContext._drain_and_barrier = _patched_drain_and_barrier

# ----------------------------------------------------------------------------
# Same walrus limitation applies to every engine instruction: split any
# instruction carrying more than _JLIM semaphore waits into preceding
# single-wait NoOps on the same engine (engines are in-order, so blocking on
# a prior NoOp is equivalent).  Done as a BIR-JSON post-pass on serialization.
# ----------------------------------------------------------------------------
import json as _json

_JLIM = 1
_orig_to_json_bytes = bass.Bass.to_json_bytes


def _split_waits_json(self) -> bytes:
    raw = _orig_to_json_bytes(self)
    d = _json.loads(raw)
    ctr = [0]

    def fix_block(blk):
        insts = blk.get("instructions")
        if insts:
            out = []
            for ins in insts:
                si = ins.get("sync_info")
                waits = (si or {}).get("on_wait") or []
                if len(waits) > _JLIM:
                    keep = waits[:_JLIM]
                    extra = waits[_JLIM:]
                    for k in range(0, len(extra), _JLIM):
                        ctr[0] += 1
                        out.append(
                            {
                                "debug": ins.get("debug", 0),
                                "engine": ins["engine"],
                                "ins": [],
                                "name": f"I-sw{ctr[0]}",
                                "opcode": "NoOp",
                                "outs": [],
                                "sync_info": {
                                    "on_wait": extra[k : k + _JLIM],
                                    "on_update": [],
                                },
                            }
                        )
                    si["on_wait"] = keep
                out.append(ins)
            blk["instructions"] = out
        for sub in blk.get("blocks", []) or []:
            fix_block(sub)

    for f in d.get("functions", []):
        for blk in f.get("blocks", []) or []:
            fix_block(blk)
    return _json.dumps(d).encode()


bass.Bass.to_json_bytes = _split_waits_json

# ----------------------------------------------------------------------------

B, T, H, E, V = 100, 50, 512, 128, 8000
G = 3 * H  # 1536
NCORES = 8
VS = V // NCORES  # 1000 vocab columns per core
KH = H // 128  # 4 K-chunks for H contraction
ROWS = B * T  # 5000 output rows
BF16 = mybir.dt.bfloat16
F32 = mybir.dt.float32
F32R = mybir.dt.float32r
AF = mybir.ActivationFunctionType
ALU = mybir.AluOpType

# stash for test.py introspection
LAST_RESULTS = None


def _const_scalar(row, name):
    row = np.asarray(row, dtype=np.float64)
    lo, hi = row.min(), row.max()
    assert hi - lo < 1e-12, f"{name} is not a constant row; fast path invalid"
    return float(row[0])


def _bf16(a):
    return np.ascontiguousarray(np.asarray(a, dtype=np.float32)).astype(
        ml_dtypes.bfloat16
    )


def _fold_layer(W, U, b, alpha, beta1, beta2):
    """Host folding of the MI-GRU cell constants.

    gate_arg = alpha*wx*uh + beta1*uh + beta2*wx + b
             = (alpha*wx + beta1) * (uh + beta2/alpha) + (b - beta1*beta2/alpha)
    """
    W, U = np.asarray(W, np.float64), np.asarray(U, np.float64)
    alpha = np.asarray(alpha, np.float64)
    beta1 = np.asarray(beta1, np.float64)
    beta2 = np.asarray(beta2, np.float64)
    b = np.asarray(b, np.float64)
    Wf = W * alpha[None, :]
    r2 = beta2 / alpha
    d = b - beta1 * beta2 / alpha
    # per-range scalars (rows are constant in this problem)
    sc = {
        "b1g": _const_scalar(beta1[: 2 * H], "beta1_g"),
        "b1c": _const_scalar(beta1[2 * H :], "beta1_c"),
        "r2g": _const_scalar(r2[: 2 * H], "r2_g"),
        "r2c": _const_scalar(r2[2 * H :], "r2_c"),
        "dg": _const_scalar(d[: 2 * H], "d_g"),
        "dc": _const_scalar(d[2 * H :], "d_c"),
    }
    return Wf.astype(np.float32), np.asarray(U, np.float32), sc


def _build_program():
    nc = bass.Bass(
        "TRN2", target_bir_lowering=False, debug=False, num_devices=NCORES
    )

    # DRAM I/O
    xsT_d = nc.dram_tensor("xsT", [T, E, B], BF16, kind="ExternalInput").ap()
    w0f_d = nc.dram_tensor("w0f", [E, G], BF16, kind="ExternalInput").ap()
    u0_d = nc.dram_tensor("u0", [KH, 128, G], F32R, kind="ExternalInput").ap()
    w1f_d = nc.dram_tensor("w1f", [KH, 128, G], F32R, kind="ExternalInput").ap()
    u1_d = nc.dram_tensor("u1", [KH, 128, G], F32R, kind="ExternalInput").ap()
    wsm_d = nc.dram_tensor("wsm", [KH, 128, VS], BF16, kind="ExternalInput").ap()
    sbr_d = nc.dram_tensor("sbr", [128, VS], F32, kind="ExternalInput").ap()
    zin_d = nc.dram_tensor("zinit", [128, KH, B], F32R, kind="ExternalInput").ap()
    sc_names = ["b1g", "b1c", "r2g", "r2c", "dg", "dc"]
    out_d = nc.dram_tensor("out", [ROWS, VS], F32, kind="ExternalOutput").ap()

    scalars = {}

    def build(tc, sc):
        nc = tc.nc
        cpool = tc.alloc_tile_pool(name="const", bufs=1)
        # persistent tensors
        ld_engs = [nc.sync, nc.gpsimd, nc.scalar]
        xs_s = cpool.tile([128, T, B], BF16, tag="xs")
        for t in range(T):
            ld_engs[t % 3].dma_start(xs_s[:, t, :], xsT_d[t])
        w0f_s = cpool.tile([128, G], BF16, tag="w0f")
        nc.sync.dma_start(w0f_s[:], w0f_d[:])
        u0_s = cpool.tile([128, KH, G], F32R, tag="u0")
        w1f_s = cpool.tile([128, KH, G], F32R, tag="w1f")
        u1_s = cpool.tile([128, KH, G], F32R, tag="u1")
        for k in range(KH):
            ld_engs[k % 3].dma_start(u0_s[:, k, :], u0_d[k])
            ld_engs[(k + 1) % 3].dma_start(w1f_s[:, k, :], w1f_d[k])
            ld_engs[(k + 2) % 3].dma_start(u1_s[:, k, :], u1_d[k])
        wsm_s = cpool.tile([128, KH, VS], BF16, tag="wsm")
        for k in range(KH):
            ld_engs[(k + 3) % 3].dma_start(wsm_s[:, k, :], wsm_d[k])
        sbr_s = cpool.tile([128, VS], F32, tag="sbr")
        nc.sync.dma_start(sbr_s[:], sbr_d[:])

        ident = cpool.tile([128, 128], F32, tag="ident")
        make_identity(nc, ident[:])

        # bias constant tiles for ACT activations (bias must be an AP)
        _bias_tiles = {}

        def bias_ap(val, parts=B):
            val = float(val)
            if val not in _bias_tiles:
                bt = cpool.tile([128, 1], F32, tag=f"bias_{len(_bias_tiles)}")
                nc.vector.memset(bt[:], val)
                _bias_tiles[val] = bt
            return _bias_tiles[val][:parts]

        h1T_all = cpool.tile([128, KH, ROWS], BF16, tag="h1T_all")

        # initial states (zeros)
        h0_s = cpool.tile([B, H], F32, tag="h0_init")
        h1_s = cpool.tile([B, H], F32, tag="h1_init")
        h0T = cpool.tile([128, KH, B], F32R, tag="h0T_init")
        h1T0 = cpool.tile([128, KH, B], F32R, tag="h1T_init")
        nc.vector.memset(h0_s[:], 0.0)
        nc.vector.memset(h1_s[:], 0.0)
        nc.sync.dma_start(h0T[:], zin_d[:])
        nc.sync.dma_start(h1T0[:], zin_d[:])

        # pools
        psA = tc.alloc_tile_pool(name="psA", bufs=1, space="PSUM")
        psU = tc.alloc_tile_pool(name="psU", bufs=5, space="PSUM")
        sb2 = tc.alloc_tile_pool(name="sb2", bufs=1)
        sb3 = tc.alloc_tile_pool(name="sb3", bufs=2)

        def transpose_hT(src, dst_ap, tagbase, dst2_ap=None):
            """src [B, H] f32 -> dst [128, KH, B]: 4 PE transposes into one
            PSUM bank, then a single merged copy (and optional bf16 copy)."""
            pst = psU.tile([128, KH, B], F32, tag="psU")
            for k in range(KH):
                nc.tensor.transpose(
                    pst[:, k, :], src[:, k * 128 : (k + 1) * 128], ident[:B, :B]
                )
            nc.vector.tensor_copy(dst_ap, pst[:, :, :])
            if dst2_ap is not None:
                nc.vector.tensor_copy(dst2_ap, pst[:, :, :])

        def cell(
            t, layer, xT_stationary, x_kchunks, Wf_s, U_s, h_prev, hT_prev_fn, sc_l
        ):
            """One MI-GRU cell. Returns (new_h sbuf [B,H] bf16, hT_new_fn)."""
            lt = f"l{layer}"
            # --- A = x @ Wf (+beta1) ---
            psa = psA.tile([B, G], F32, tag="psA")
            for n in range(3):
                ns = slice(n * 512, (n + 1) * 512)
                for ki in range(x_kchunks):
                    nc.tensor.matmul(
                        psa[:, ns],
                        xT_stationary(ki),
                        Wf_s[:, ki, ns] if x_kchunks > 1 else Wf_s[:, ns],
                        start=(ki == 0),
                        stop=(ki == x_kchunks - 1),
                    )
            A_s = sb2.tile([B, G], F32, tag=f"A{lt}")
            # r-part move first (chain-critical), then z+c parts
            nc.scalar.activation(
                A_s[:, :512], psa[:, :512], AF.Identity, bias=bias_ap(sc_l["b1g"])
            )
            nc.scalar.activation(
                A_s[:, 512:], psa[:, 512:], AF.Identity, bias=bias_ap(sc_l["b1g"])
            )  # cols 512:1024 use b1g, 1024: use b1c (equal here; host asserts)

            # --- r gate (chain critical) ---
            psr = psU.tile([B, 512], F32, tag="psU")
            for k in range(KH):
                nc.tensor.matmul(
                    psr[:],
                    hT_prev_fn(k),
                    U_s[:, k, 0:512],
                    start=(k == 0),
                    stop=(k == KH - 1),
                )
            m_r = sb2.tile([B, 512], F32, tag=f"mr{lt}")
            nc.vector.scalar_tensor_tensor(
                m_r[:], psr[:], sc_l["r2g"], A_s[:, :512], ALU.add, ALU.mult
            )
            r = sb2.tile([B, 512], F32, tag=f"r{lt}")
            nc.scalar.activation(r[:], m_r[:], AF.Sigmoid, bias=bias_ap(sc_l["dg"]))

            # --- z gate (off critical path) ---
            psz = psU.tile([B, 512], F32, tag="psU")
            for k in range(KH):
                nc.tensor.matmul(
                    psz[:],
                    hT_prev_fn(k),
                    U_s[:, k, 512:1024],
                    start=(k == 0),
                    stop=(k == KH - 1),
                )
            m_z = sb2.tile([B, 512], F32, tag=f"mz{lt}")
            nc.vector.scalar_tensor_tensor(
                m_z[:], psz[:], sc_l["r2g"], A_s[:, 512:1024], ALU.add, ALU.mult
            )
            z = sb2.tile([B, 512], F32, tag=f"z{lt}")
            nc.scalar.activation(z[:], m_z[:], AF.Sigmoid, bias=bias_ap(sc_l["dg"]))
            # zh = z * h_prev (off critical path)
            zh = sb2.tile([B, 512], BF16, tag=f"zh{lt}")
            nc.gpsimd.tensor_mul(zh[:], z[:], h_prev[:])

            # --- candidate ---
            rh = sb2.tile([B, 512], F32, tag=f"rh{lt}")
            nc.vector.tensor_mul(rh[:], r[:], h_prev[:])
            rhT = sb2.tile([128, KH, B], F32R, tag="rhT", bufs=2)
            transpose_hT(rh, rhT[:, :, :], f"rhT{lt}")
            psc = psU.tile([B, 512], F32, tag="psU")
            for k in range(KH):
                nc.tensor.matmul(
                    psc[:],
                    rhT[:, k, :],
                    U_s[:, k, 1024:1536],
                    start=(k == 0),
                    stop=(k == KH - 1),
                )
            m_c = sb2.tile([B, 512], F32, tag=f"mc{lt}")
            nc.vector.scalar_tensor_tensor(
                m_c[:], psc[:], sc_l["r2c"], A_s[:, 1024:], ALU.add, ALU.mult
            )
            cc = sb2.tile([B, 512], BF16, tag=f"c{lt}")
            nc.scalar.activation(cc[:], m_c[:], AF.Tanh, bias=bias_ap(sc_l["dc"]))

            # --- new_h = z*h + (1-z)*c  =  zh - (z-1)*c ---
            q = sb2.tile([B, 512], BF16, tag=f"q{lt}")
            nc.vector.scalar_tensor_tensor(
                q[:], z[:], 1.0, cc[:], ALU.subtract, ALU.mult
            )
            nh = sb3.tile([B, H], F32, tag=f"h{lt}")
            nc.gpsimd.tensor_sub(nh[:], zh[:], q[:])
            return nh

        sc0, sc1 = sc["l0"], sc["l1"]
        for t in range(T):
            # ---- cell 0 ----
            nh0 = cell(
                t,
                0,
                lambda ki, t=t: xs_s[:, t, :],
                1,
                w0f_s,
                u0_s,
                h0_s,
                lambda k, h0T=h0T: h0T[:, k, :],
                sc0,
            )
            h0T_new = sb2.tile([128, KH, B], F32R, tag="h0T", bufs=2)
            transpose_hT(nh0, h0T_new[:, :, :], "h0T")
            # ---- cell 1 ----
            if t == 0:
                hT1fn = lambda k: h1T0[:, k, :]
            else:
                h1T_prev_t = h1T_rec
                hT1fn = lambda k: h1T_prev_t[:, k, :]
            nh1 = cell(
                t,
                1,
                lambda ki: h0T_new[:, ki, :],
                KH,
                w1f_s,
                u1_s,
                h1_s,
                hT1fn,
                sc1,
            )
            h1T_rec = sb2.tile([128, KH, B], F32R, tag="h1T", bufs=2)
            transpose_hT(
                nh1,
                h1T_rec[:, :, :],
                "h1T",
                dst2_ap=h1T_all[:, :, t * B : (t + 1) * B],
            )
            h0_s, h1_s, h0T = nh0, nh1, h0T_new

        # ---- projection: out[rows, VS] = h1_all @ wsm + sb ----
        dma_engines = [nc.sync, nc.gpsimd, nc.scalar]
        NB = 2  # two 500-wide column banks
        NBW = VS // NB
        n_mtiles = (ROWS + 127) // 128
        for m in range(n_mtiles):
            r0 = m * 128
            mrows = min(128, ROWS - r0)
            for nb in range(NB):
                ns = slice(nb * NBW, (nb + 1) * NBW)
                psp = psU.tile([128, NBW], F32, tag="psU")
                for k in range(KH):
                    nc.tensor.matmul(
                        psp[:mrows, :],
                        h1T_all[:, k, r0 : r0 + mrows],
                        wsm_s[:, k, ns],
                        start=(k == 0),
                        stop=(k == KH - 1),
                    )
                lo = sb3.tile([128, NBW], F32, tag="lout")
                nc.vector.tensor_add(lo[:mrows, :], psp[:mrows, :], sbr_s[:mrows, ns])
                eng = dma_engines[(m * NB + nb) % len(dma_engines)]
                eng.dma_start(out_d[r0 : r0 + mrows, ns], lo[:mrows, :])

        for p in (sb3, sb2, psU, psA, cpool):
            p.release()

    return nc, build, scalars


def kernel(**inputs):
    global LAST_RESULTS
    inp = {k: np.asarray(v) for k, v in inputs.items()}

    # ---- host prep ----
    xs = np.asarray(inp["embedding"], np.float32)[np.asarray(inp["input_data"])]
    # xs: [B, T, E] -> [T, E, B]
    xsT = np.ascontiguousarray(xs.transpose(1, 2, 0))

    W0f, U0, sc0 = _fold_layer(
        inp["W0"], inp["U0"], inp["b0"], inp["alpha0"], inp["beta1_0"], inp["beta2_0"]
    )
    W1f, U1, sc1 = _fold_layer(
        inp["W1"], inp["U1"], inp["b1"], inp["alpha1"], inp["beta1_1"], inp["beta2_1"]
    )
    for sc in (sc0, sc1):
        assert abs(sc["b1g"] - sc["b1c"]) < 1e-12, "split A-move biases needed"

    u0c = np.ascontiguousarray(U0.reshape(KH, 128, G))
    w1c = np.ascontiguousarray(W1f.reshape(KH, 128, G))
    u1c = np.ascontiguousarray(U1.reshape(KH, 128, G))

    wsm = np.asarray(inp["softmax_w"], np.float32)  # [H, V]
    sb = np.asarray(inp["softmax_b"], np.float32)  # [V]

    nc, build, _ = _build_program()
    with tile.TileContext(nc) as tc:
        build(tc, {"l0": sc0, "l1": sc1})

    base_map = {
        "zinit": np.zeros((128, KH, B), np.float32),
        "xsT": _bf16(xsT),
        "w0f": _bf16(W0f),
        "u0": np.ascontiguousarray(u0c, dtype=np.float32),
        "w1f": np.ascontiguousarray(w1c, dtype=np.float32),
        "u1": np.ascontiguousarray(u1c, dtype=np.float32),
    }
    in_maps = []
    for c in range(NCORES):
        vs = slice(c * VS, (c + 1) * VS)
        m = dict(base_map)
        m["wsm"] = _bf16(np.ascontiguousarray(wsm[:, vs]).reshape(KH, 128, VS))
        m["sbr"] = np.ascontiguousarray(
            np.tile(sb[vs][None, :], (128, 1)).astype(np.float32)
        )
        in_maps.append(m)

    from concourse.bass_utils import run_bass_kernel_spmd

    trace = bool(int(os.environ.get("KERNEL_TRACE", "0")))
    res = run_bass_kernel_spmd(
        nc, in_maps, core_ids=list(range(NCORES)), trace=trace
    )
    LAST_RESULTS = res

    # ---- assemble: concat vocab slices, reorder rows (t-major -> b-major) ----
    logits_tb = np.concatenate(
        [res.results[c]["out"] for c in range(NCORES)], axis=1
    )  # [T*B, V]
    logits = (
        logits_tb.reshape(T, B, V).transpose(1, 0, 2).reshape(B * T, V)
    )
    return np.ascontiguousarray(logits.astype(np.float32))

